# revision 58
# baseline (speedup 1.0000x reference)
"""Trainium2 Bass kernel for AttentionGuidedMaskStrategy (topk_masking).

Per batch b and side (a->mask_b, b->mask_a):
  key[j]  = sum_i qmask[i] * attn[b, i, j]          (PE matmul, K=512 in 4 chunks)
  rank[j] = #{j' : key[j'] < key[j]}                (DVE compare + fused accum)
  mask[j] = (rank[j] + 1 <= 0.3 * n_nonpad_keys)    (exact int() truncation semantics)
  out[j,:] = mask[j] ? mask_embedding : embed[b,j,:] (copy_predicated)

Data parallel over 8 NeuronCores: 8 batches per core, no collectives.

Engine layout (v2):
  sync  (HWDGE ring A): attn loads, 1 MB each
  scalar(HWDGE ring B): embed loads + output stores, 0.5 MB each
  PE:     key-sum matmuls (fp32) + tiny vt transposes
  gpsimd: partition_broadcast of the key-sum row (replaces PE broadcast matmul)
  DVE:    rank compares (reading SBUF), mask compare, blend
"""

import sys

for _p in ("/opt/trn_rl_repo",):
    if _p not in sys.path:
        sys.path.insert(0, _p)

import numpy as np
from contextlib import ExitStack

from concourse import bacc, bass, mybir
from concourse.bass_utils import run_bass_kernel_spmd
from concourse.tile import TileContext, add_dep_helper

N_CORES = 8
B_LOC = 8      # 64 batches / 8 cores
L = 512        # La == Lb
E = 256
P = 128
NKC = L // P   # 4 chunks of 128
F32 = mybir.dt.float32
U8 = mybir.dt.uint8
OP = mybir.AluOpType


def _build() -> bass.Bass:
    nc = bacc.Bacc(None, target_bir_lowering=False)

    attn_a = nc.declare_dram_parameter("attn_a", [B_LOC, L, L], F32, isOutput=False)
    attn_b = nc.declare_dram_parameter("attn_b", [B_LOC, L, L], F32, isOutput=False)
    embed_a = nc.declare_dram_parameter("embed_a", [B_LOC, L, E], F32, isOutput=False)
    embed_b = nc.declare_dram_parameter("embed_b", [B_LOC, L, E], F32, isOutput=False)
    memb = nc.declare_dram_parameter("mask_embedding", [1, E], F32, isOutput=False)
    a_pad = nc.declare_dram_parameter("a_padding_mask", [B_LOC, L], U8, isOutput=False)
    b_pad = nc.declare_dram_parameter("b_padding_mask", [B_LOC, L], U8, isOutput=False)
    out_b = nc.declare_dram_parameter("out_b", [B_LOC, L, E], F32, isOutput=True)
    out_a = nc.declare_dram_parameter("out_a", [B_LOC, L, E], F32, isOutput=True)

    with TileContext(nc) as tc, ExitStack() as ctx:
        const = ctx.enter_context(tc.tile_pool(name="const", bufs=1))

        ones_k1 = const.tile([1, P], F32)       # lhsT for 1x1 transposes
        nc.vector.memset(ones_k1[:], 1.0)
        ones_k128 = const.tile([P, 1], F32)     # lhsT for partition-sum
        nc.vector.memset(ones_k128[:], 1.0)

        # warm the sync ring + HBM path before the first big attn transfer
        warm = const.tile([1, L], F32)
        nc.sync.dma_start(out=warm[:], in_=attn_a[0, 0, :])

        # setup DMAs go on the scalar ring so they don't entangle with the
        # attn stream's completion semaphores on the sync ring.  The padding
        # masks are loaded row-major (clean 512B lines; a byte-granular
        # gather DMA here costs ~5us of pure descriptor overhead) and
        # transposed to [p, side, b, kc] on the PE via an 8x8 identity.
        pad_sb = const.tile([B_LOC, 2, L], U8)
        nc.scalar.dma_start(out=pad_sb[:, 0], in_=a_pad[:, :])
        nc.scalar.dma_start(out=pad_sb[:, 1], in_=b_pad[:, :])
        memb_sb = const.tile([1, E], F32)
        nc.scalar.dma_start(out=memb_sb[:], in_=memb[:, :])

        padf = const.tile([B_LOC, 2, L], mybir.dt.bfloat16)
        nc.vector.tensor_scalar(padf[:], pad_sb[:], 0.0, None, op0=OP.is_equal)
        it8 = const.tile([B_LOC, B_LOC], mybir.dt.int32)
        nc.gpsimd.iota(it8[:], pattern=[[1, B_LOC]], base=0, channel_multiplier=-1)
        ident8 = const.tile([B_LOC, B_LOC], mybir.dt.bfloat16)
        nc.vector.tensor_scalar(ident8[:], it8[:], 0, None, op0=OP.is_equal)

        psum_setup = ctx.enter_context(tc.tile_pool(name="psum_setup", bufs=1,
                                                    space="PSUM"))
        qm_ps = psum_setup.tile([P, 2, B_LOC, NKC], F32, tag="qm_ps")
        for si in range(2):
            for kc in range(NKC):
                nc.tensor.matmul(qm_ps[:, si, :, kc],
                                 padf[:, si, kc * P:(kc + 1) * P], ident8[:],
                                 start=True, stop=True)
        qm_all = const.tile([P, 2, B_LOC, NKC], F32)
        nc.vector.tensor_copy(qm_all[:], qm_ps[:])

        # per-batch non-padded key counts, per side: sum over partitions.
        # row block 0: masks over b keys (k from len_b); block 1: over a keys
        cnt_ps = psum_setup.tile([1, 2, B_LOC, NKC], F32, tag="cnt_ps")
        nc.tensor.matmul(cnt_ps[:, 0], ones_k128[:], qm_all[:, 1], start=True, stop=True)
        nc.tensor.matmul(cnt_ps[:, 1], ones_k128[:], qm_all[:, 0], start=True, stop=True)

        qrow = const.tile([1, 2, B_LOC], F32)
        nc.vector.tensor_reduce(qrow[:], cnt_ps[:], axis=mybir.AxisListType.X, op=OP.add)
        # km1 = 0.3 * count - 1:  mask condition rank < int(q) <=> rank <= q-1.
        # (0.3*count in f32 matches jnp's ratio * count.astype(f32) bit-exactly.)
        nc.vector.tensor_scalar_mul(qrow[:], qrow[:], 0.3)
        km1_row = const.tile([1, 2, B_LOC], F32)
        nc.vector.tensor_scalar_sub(km1_row[:], qrow[:], 1.0)

        # broadcast mask_embedding and km1 down all 128 partitions on gpsimd
        memb_bc = const.tile([P, E], F32)
        nc.gpsimd.partition_broadcast(memb_bc[:], memb_sb[:])
        km1_bc = const.tile([P, 2, B_LOC], F32)
        nc.gpsimd.partition_broadcast(
            km1_bc[:].rearrange("p s b -> p (s b)"),
            km1_row[:].rearrange("a s b -> a (s b)"))

        attn_pool = ctx.enter_context(tc.tile_pool(name="attn", bufs=10))
        emb_pool = ctx.enter_context(tc.tile_pool(name="emb", bufs=6))
        row_pool = ctx.enter_context(tc.tile_pool(name="row", bufs=4))
        vbc_pool = ctx.enter_context(tc.tile_pool(name="vbc", bufs=3))
        scr_pool = ctx.enter_context(tc.tile_pool(name="scr", bufs=2))
        v_psum = ctx.enter_context(tc.tile_pool(name="v_ps", bufs=3, space="PSUM"))
        vt_psum = ctx.enter_context(tc.tile_pool(name="vt_ps", bufs=3, space="PSUM"))

        # (attn, qmask side index, embed in/out over keys, km1 row-block)
        sides = [
            (attn_a, 0, embed_b, out_b, 0),
            (attn_b, 1, embed_a, out_a, 1),
        ]
        rows = [(b,) + s for b in range(B_LOC) for s in sides]

        def emit_sums(r):
            """key sums: v[j] = sum_i qmask[i] attn[i, j] (4 acc. matmuls)"""
            b, attn, qsi, emb, outp, si = rows[r]
            if r < 2:
                # chunked loads into separate tiles for the first rows: the
                # first sum matmul starts as soon as chunk 0 lands instead of
                # waiting out the full 1 MB transfer
                chunks = []
                for kc in range(NKC):
                    atk = attn_pool.tile([P, L], F32, tag=f"attn_c{kc}")
                    # alternate rings: two chunk transfers in flight at once
                    # while the DMA path is still cold
                    ring_c = nc.sync if kc % 2 == 0 else nc.scalar
                    ring_c.dma_start(out=atk[:],
                                     in_=attn[b, kc * P:(kc + 1) * P, :])
                    chunks.append(atk[:])
            else:
                at = attn_pool.tile([P, NKC, L], F32, tag="attn")
                ring_a = nc.sync if r % 2 == 0 else nc.scalar
                ring_a.dma_start(
                    out=at[:], in_=attn.rearrange("b (kc p) l -> b p kc l", p=P)[b])
                chunks = [at[:, kc] for kc in range(NKC)]
            et = emb_pool.tile([P, NKC, E], F32, tag="emb")
            ring_e = nc.scalar if r % 2 == 0 else nc.sync
            ring_e.dma_start(
                out=et[:], in_=emb.rearrange("b (kc p) e -> b p kc e", p=P)[b])
            v_ps = v_psum.tile([1, L], F32, tag="v")
            for kc in range(NKC):
                nc.tensor.matmul(v_ps[:], qm_all[:, qsi, b, kc:kc + 1], chunks[kc],
                                 start=(kc == 0), stop=(kc == NKC - 1))
            vrow = row_pool.tile([1, L], F32, tag="vrow")
            nc.scalar.copy(vrow[:], v_ps[:])
            # vbc here (not in emit_rank) so gpsimd's in-order queue runs
            # vbc(r) ahead of mask(r-1) and never stalls the DVE rank pass
            vbc = vbc_pool.tile([P, L], F32, tag="vbc")
            nc.gpsimd.partition_broadcast(vbc[:], vrow[:])
            return vrow, vbc, et

        def emit_rank(r, vrow, vbc):
            b, attn, qsi, emb, outp, si = rows[r]
            vt_ps = vt_psum.tile([P, NKC], F32, tag="vt")
            for kc in range(NKC):
                nc.tensor.matmul(vt_ps[:, kc:kc + 1], vrow[:, kc * P:(kc + 1) * P],
                                 ones_k1[:, 0:1], start=True, stop=True)
            # rank[p, kc] = #{j : v[j] < vT[p, kc]}
            rank4 = row_pool.tile([P, NKC], F32, tag="rank")
            for kc in range(NKC):
                scr = scr_pool.tile([P, L], U8, tag="scr")
                nc.vector.tensor_scalar(
                    scr[:], vbc[:], vt_ps[:, kc:kc + 1], None,
                    op0=OP.is_lt, op1=OP.add, accum_out=rank4[:, kc:kc + 1])

            # mask = rank <= q - 1 (integer-valued f32 compare, exact) on the
            # lightly-loaded gpsimd engine
            mask4 = row_pool.tile([P, NKC], U8, tag="mask")
            nc.gpsimd.tensor_scalar(mask4[:], rank4[:], km1_bc[:, si, b:b + 1],
                                    None, op0=OP.is_le)
            return mask4

        def emit_blend(r, et, mask4):
            b, attn, qsi, emb, outp, si = rows[r]
            # blend in place (one fused predicated copy), store in one DMA.
            # Late rows store on the (by then idle) sync ring.
            nc.vector.copy_predicated(
                et[:], mask4[:].unsqueeze(2).to_broadcast([P, NKC, E]),
                memb_bc[:].unsqueeze(1).to_broadcast([P, NKC, E]))
            # late stores go to the sync HWDGE ring (idle once the attn
            # stream is issued) so SWDGE descriptor-gen never delays the
            # tail-critical gpsimd mask/broadcast ops
            ring_s = nc.sync if r >= 14 else nc.gpsimd
            ring_s.dma_start(
                out=outp.rearrange("b (kc p) e -> b p kc e", p=P)[b], in_=et[:])

        def emit_tail(r, vrow, vbc, et):
            """Last row: rank/mask/blend/store split in two halves so the
            first half's store overlaps the second half's compute."""
            b, attn, qsi, emb, outp, si = rows[r]
            vt_ps = vt_psum.tile([P, NKC], F32, tag="vt")
            for kc in range(NKC):
                nc.tensor.matmul(vt_ps[:, kc:kc + 1], vrow[:, kc * P:(kc + 1) * P],
                                 ones_k1[:, 0:1], start=True, stop=True)
            rank4 = row_pool.tile([P, NKC], F32, tag="rank")
            for kc in range(NKC):
                scr = scr_pool.tile([P, L], U8, tag="scr")
                nc.vector.tensor_scalar(
                    scr[:], vbc[:], vt_ps[:, kc:kc + 1], None,
                    op0=OP.is_lt, op1=OP.add, accum_out=rank4[:, kc:kc + 1])
            out2 = outp.rearrange("b (h k2 p) e -> (b h) p k2 e", h=2, p=P)
            for h in range(2):
                maskh = row_pool.tile([P, 2], U8, tag=f"maskh{h}")
                nc.gpsimd.tensor_scalar(maskh[:], rank4[:, 2 * h:2 * h + 2],
                                        km1_bc[:, si, b:b + 1], None, op0=OP.is_le)
                nc.vector.copy_predicated(
                    et[:, 2 * h:2 * h + 2],
                    maskh[:].unsqueeze(2).to_broadcast([P, 2, E]),
                    memb_bc[:].unsqueeze(1).to_broadcast([P, 2, E]))
                nc.sync.dma_start(out=out2[b * 2 + h], in_=et[:, 2 * h:2 * h + 2])

        # 3-stage software pipeline: sums(r) | rank(r-1) | blend(r-2).
        # blend(r-2) is emitted before rank(r-1) so the DVE always has
        # ready work while PE finishes sums(r)/vt(r-1).
        state = {}
        nrows = len(rows)
        for r in range(nrows):
            state[r] = {}
            state[r]["vrow"], state[r]["vbc"], state[r]["et"] = emit_sums(r)
            if r <= 1:
                # head: start the rank pipeline immediately for the first two
                # rows (small one-time PE waits on the vrow copies, but the
                # DVE stream fills much sooner)
                state[r]["mask"] = emit_rank(r, state[r]["vrow"], state[r]["vbc"])
            if r >= 2:
                emit_blend(r - 2, state[r - 2]["et"], state[r - 2]["mask"])
            if r >= 3:
                state[r - 1]["mask"] = emit_rank(
                    r - 1, state[r - 1]["vrow"], state[r - 1]["vbc"])
        # blend(14) is emitted before rank(15): its inputs are ready while
        # rank(15) still waits on the last (DMA-gated) attn row, so the DVE
        # keeps working through the drain
        emit_blend(nrows - 2, state[nrows - 2]["et"], state[nrows - 2]["mask"])
        emit_tail(nrows - 1, state[nrows - 1]["vrow"], state[nrows - 1]["vbc"],
                  state[nrows - 1]["et"])

    nc.compile()
    return nc


_NC_CACHE = None


def _get_nc() -> bass.Bass:
    global _NC_CACHE
    if _NC_CACHE is None:
        _NC_CACHE = _build()
    return _NC_CACHE


def _run(inputs, trace=False):
    nc = _get_nc()
    in_maps = []
    for c in range(N_CORES):
        sl = slice(c * B_LOC, (c + 1) * B_LOC)
        in_maps.append({
            "attn_a": np.ascontiguousarray(np.asarray(inputs["attn_a"])[sl]),
            "attn_b": np.ascontiguousarray(np.asarray(inputs["attn_b"])[sl]),
            "embed_a": np.ascontiguousarray(np.asarray(inputs["embed_a"])[sl]),
            "embed_b": np.ascontiguousarray(np.asarray(inputs["embed_b"])[sl]),
            "mask_embedding": np.asarray(inputs["mask_embedding"]),
            "a_padding_mask": np.ascontiguousarray(
                np.asarray(inputs["a_padding_mask"])[sl]).view(np.uint8),
            "b_padding_mask": np.ascontiguousarray(
                np.asarray(inputs["b_padding_mask"])[sl]).view(np.uint8),
        })
    res = run_bass_kernel_spmd(nc, in_maps, core_ids=list(range(N_CORES)), trace=trace)
    out_b = np.concatenate([res.results[c]["out_b"] for c in range(N_CORES)], axis=0)
    out_a = np.concatenate([res.results[c]["out_a"] for c in range(N_CORES)], axis=0)
    return (out_b, out_a), res


def kernel(**inputs):
    outs, _ = _run(inputs, trace=False)
    return outs


# revision 59
# speedup vs baseline: 1.0462x; 1.0462x over previous
"""Trainium2 Bass kernel for AttentionGuidedMaskStrategy (topk_masking).

Per batch b and side (a->mask_b, b->mask_a):
  key[j]  = sum_i qmask[i] * attn[b, i, j]          (PE matmul, K=512 in 4 chunks)
  rank[j] = #{j' : key[j'] < key[j]}                (DVE compare + fused accum)
  mask[j] = (rank[j] + 1 <= 0.3 * n_nonpad_keys)    (exact int() truncation semantics)
  out[j,:] = mask[j] ? mask_embedding : embed[b,j,:] (copy_predicated)

Data parallel over 8 NeuronCores: 8 batches per core, no collectives.

Engine layout (v2):
  sync  (HWDGE ring A): attn loads, 1 MB each
  scalar(HWDGE ring B): embed loads + output stores, 0.5 MB each
  PE:     key-sum matmuls (fp32) + tiny vt transposes
  gpsimd: partition_broadcast of the key-sum row (replaces PE broadcast matmul)
  DVE:    rank compares (reading SBUF), mask compare, blend
"""

import sys

for _p in ("/opt/trn_rl_repo",):
    if _p not in sys.path:
        sys.path.insert(0, _p)

import numpy as np
from contextlib import ExitStack

from concourse import bacc, bass, mybir
from concourse.bass_utils import run_bass_kernel_spmd
from concourse.tile import TileContext, add_dep_helper

N_CORES = 8
B_LOC = 8      # 64 batches / 8 cores
L = 512        # La == Lb
E = 256
P = 128
NKC = L // P   # 4 chunks of 128
F32 = mybir.dt.float32
U8 = mybir.dt.uint8
OP = mybir.AluOpType


def _build() -> bass.Bass:
    nc = bacc.Bacc(None, target_bir_lowering=False)

    attn_a = nc.declare_dram_parameter("attn_a", [B_LOC, L, L], F32, isOutput=False)
    attn_b = nc.declare_dram_parameter("attn_b", [B_LOC, L, L], F32, isOutput=False)
    embed_a = nc.declare_dram_parameter("embed_a", [B_LOC, L, E], F32, isOutput=False)
    embed_b = nc.declare_dram_parameter("embed_b", [B_LOC, L, E], F32, isOutput=False)
    memb = nc.declare_dram_parameter("mask_embedding", [1, E], F32, isOutput=False)
    a_pad = nc.declare_dram_parameter("a_padding_mask", [B_LOC, L], U8, isOutput=False)
    b_pad = nc.declare_dram_parameter("b_padding_mask", [B_LOC, L], U8, isOutput=False)
    out_b = nc.declare_dram_parameter("out_b", [B_LOC, L, E], F32, isOutput=True)
    out_a = nc.declare_dram_parameter("out_a", [B_LOC, L, E], F32, isOutput=True)

    with TileContext(nc) as tc, ExitStack() as ctx:
        const = ctx.enter_context(tc.tile_pool(name="const", bufs=1))

        ones_k1 = const.tile([1, P], F32)       # lhsT for 1x1 transposes
        nc.vector.memset(ones_k1[:], 1.0)
        ones_k128 = const.tile([P, 1], F32)     # lhsT for partition-sum
        nc.vector.memset(ones_k128[:], 1.0)

        # warm the sync ring + HBM path before the first big attn transfer
        warm = const.tile([1, L], F32)
        nc.sync.dma_start(out=warm[:], in_=attn_a[0, 0, :])

        # setup DMAs go on the scalar ring so they don't entangle with the
        # attn stream's completion semaphores on the sync ring.  The padding
        # masks are loaded row-major (clean 512B lines; a byte-granular
        # gather DMA here costs ~5us of pure descriptor overhead) and
        # transposed to [p, side, b, kc] on the PE via an 8x8 identity.
        pad_sb = const.tile([B_LOC, 2, L], U8)
        nc.scalar.dma_start(out=pad_sb[:, 0], in_=a_pad[:, :])
        nc.scalar.dma_start(out=pad_sb[:, 1], in_=b_pad[:, :])
        memb_sb = const.tile([1, E], F32)
        nc.scalar.dma_start(out=memb_sb[:], in_=memb[:, :])

        padf = const.tile([B_LOC, 2, L], mybir.dt.bfloat16)
        nc.vector.tensor_scalar(padf[:], pad_sb[:], 0.0, None, op0=OP.is_equal)
        it8 = const.tile([B_LOC, B_LOC], mybir.dt.int32)
        nc.gpsimd.iota(it8[:], pattern=[[1, B_LOC]], base=0, channel_multiplier=-1)
        ident8 = const.tile([B_LOC, B_LOC], mybir.dt.bfloat16)
        nc.vector.tensor_scalar(ident8[:], it8[:], 0, None, op0=OP.is_equal)

        psum_setup = ctx.enter_context(tc.tile_pool(name="psum_setup", bufs=1,
                                                    space="PSUM"))
        qm_ps = psum_setup.tile([P, 2, B_LOC, NKC], F32, tag="qm_ps")
        for si in range(2):
            for kc in range(NKC):
                nc.tensor.matmul(qm_ps[:, si, :, kc],
                                 padf[:, si, kc * P:(kc + 1) * P], ident8[:],
                                 start=True, stop=True)
        qm_all = const.tile([P, 2, B_LOC, NKC], F32)
        nc.vector.tensor_copy(qm_all[:], qm_ps[:])

        # per-batch non-padded key counts, per side: sum over partitions.
        # row block 0: masks over b keys (k from len_b); block 1: over a keys
        cnt_ps = psum_setup.tile([1, 2, B_LOC, NKC], F32, tag="cnt_ps")
        nc.tensor.matmul(cnt_ps[:, 0], ones_k128[:], qm_all[:, 1], start=True, stop=True)
        nc.tensor.matmul(cnt_ps[:, 1], ones_k128[:], qm_all[:, 0], start=True, stop=True)

        qrow = const.tile([1, 2, B_LOC], F32)
        nc.vector.tensor_reduce(qrow[:], cnt_ps[:], axis=mybir.AxisListType.X, op=OP.add)
        # km1 = 0.3 * count - 1:  mask condition rank < int(q) <=> rank <= q-1.
        # (0.3*count in f32 matches jnp's ratio * count.astype(f32) bit-exactly.)
        nc.vector.tensor_scalar_mul(qrow[:], qrow[:], 0.3)
        km1_row = const.tile([1, 2, B_LOC], F32)
        nc.vector.tensor_scalar_sub(km1_row[:], qrow[:], 1.0)

        # broadcast mask_embedding and km1 down all 128 partitions on gpsimd
        memb_bc = const.tile([P, E], F32)
        nc.gpsimd.partition_broadcast(memb_bc[:], memb_sb[:])
        km1_bc = const.tile([P, 2, B_LOC], F32)
        nc.gpsimd.partition_broadcast(
            km1_bc[:].rearrange("p s b -> p (s b)"),
            km1_row[:].rearrange("a s b -> a (s b)"))

        attn_pool = ctx.enter_context(tc.tile_pool(name="attn", bufs=10))
        emb_pool = ctx.enter_context(tc.tile_pool(name="emb", bufs=6))
        row_pool = ctx.enter_context(tc.tile_pool(name="row", bufs=4))
        vbc_pool = ctx.enter_context(tc.tile_pool(name="vbc", bufs=3))
        scr_pool = ctx.enter_context(tc.tile_pool(name="scr", bufs=2))
        v_psum = ctx.enter_context(tc.tile_pool(name="v_ps", bufs=3, space="PSUM"))
        vt_psum = ctx.enter_context(tc.tile_pool(name="vt_ps", bufs=3, space="PSUM"))

        # (attn, qmask side index, embed in/out over keys, km1 row-block)
        sides = [
            (attn_a, 0, embed_b, out_b, 0),
            (attn_b, 1, embed_a, out_a, 1),
        ]
        rows = [(b,) + s for b in range(B_LOC) for s in sides]

        def emit_sums(r):
            """key sums: v[j] = sum_i qmask[i] attn[i, j] (4 acc. matmuls)"""
            b, attn, qsi, emb, outp, si = rows[r]
            if r < 2:
                # chunked loads into separate tiles for the first rows: the
                # first sum matmul starts as soon as chunk 0 lands instead of
                # waiting out the full 1 MB transfer
                chunks = []
                for kc in range(NKC):
                    atk = attn_pool.tile([P, L], F32, tag=f"attn_c{kc}")
                    # alternate rings: two chunk transfers in flight at once
                    # while the DMA path is still cold
                    ring_c = nc.sync if kc % 2 == 0 else nc.scalar
                    ring_c.dma_start(out=atk[:],
                                     in_=attn[b, kc * P:(kc + 1) * P, :])
                    chunks.append(atk[:])
            else:
                at = attn_pool.tile([P, NKC, L], F32, tag="attn")
                ring_a = nc.sync if r % 2 == 0 else nc.scalar
                ring_a.dma_start(
                    out=at[:], in_=attn.rearrange("b (kc p) l -> b p kc l", p=P)[b])
                chunks = [at[:, kc] for kc in range(NKC)]
            et = emb_pool.tile([P, NKC, E], F32, tag="emb")
            ring_e = nc.scalar if r % 2 == 0 else nc.sync
            ring_e.dma_start(
                out=et[:], in_=emb.rearrange("b (kc p) e -> b p kc e", p=P)[b])
            v_ps = v_psum.tile([1, L], F32, tag="v")
            for kc in range(NKC):
                nc.tensor.matmul(v_ps[:], qm_all[:, qsi, b, kc:kc + 1], chunks[kc],
                                 start=(kc == 0), stop=(kc == NKC - 1))
            vrow = row_pool.tile([1, L], F32, tag="vrow")
            nc.scalar.copy(vrow[:], v_ps[:])
            # vbc here (not in emit_rank) so gpsimd's in-order queue runs
            # vbc(r) ahead of mask(r-1) and never stalls the DVE rank pass
            vbc = vbc_pool.tile([P, L], F32, tag="vbc")
            nc.gpsimd.partition_broadcast(vbc[:], vrow[:])
            return vrow, vbc, et

        def emit_rank(r, vrow, vbc):
            b, attn, qsi, emb, outp, si = rows[r]
            vt_ps = vt_psum.tile([P, NKC], F32, tag="vt")
            for kc in range(NKC):
                nc.tensor.matmul(vt_ps[:, kc:kc + 1], vrow[:, kc * P:(kc + 1) * P],
                                 ones_k1[:, 0:1], start=True, stop=True)
            # rank[p, kc] = #{j : v[j] < vT[p, kc]}
            rank4 = row_pool.tile([P, NKC], F32, tag="rank")
            for kc in range(NKC):
                scr = scr_pool.tile([P, L], U8, tag="scr")
                nc.vector.tensor_scalar(
                    scr[:], vbc[:], vt_ps[:, kc:kc + 1], None,
                    op0=OP.is_lt, op1=OP.add, accum_out=rank4[:, kc:kc + 1])

            # mask = rank <= q - 1 (integer-valued f32 compare, exact) on the
            # lightly-loaded gpsimd engine
            mask4 = row_pool.tile([P, NKC], U8, tag="mask")
            nc.gpsimd.tensor_scalar(mask4[:], rank4[:], km1_bc[:, si, b:b + 1],
                                    None, op0=OP.is_le)
            return mask4

        def emit_blend(r, et, mask4):
            b, attn, qsi, emb, outp, si = rows[r]
            # blend in place (one fused predicated copy), store in one DMA.
            # Late rows store on the (by then idle) sync ring.
            nc.vector.copy_predicated(
                et[:], mask4[:].unsqueeze(2).to_broadcast([P, NKC, E]),
                memb_bc[:].unsqueeze(1).to_broadcast([P, NKC, E]))
            # late stores go to the sync HWDGE ring (idle once the attn
            # stream is issued) so SWDGE descriptor-gen never delays the
            # tail-critical gpsimd mask/broadcast ops
            ring_s = nc.sync if r >= 14 else nc.gpsimd
            ring_s.dma_start(
                out=outp.rearrange("b (kc p) e -> b p kc e", p=P)[b], in_=et[:])

        def emit_tail(r, vrow, vbc, et):
            """Last row: rank/mask/blend/store split in two halves so the
            first half's store overlaps the second half's compute."""
            b, attn, qsi, emb, outp, si = rows[r]
            vt_ps = vt_psum.tile([P, NKC], F32, tag="vt")
            for kc in range(NKC):
                nc.tensor.matmul(vt_ps[:, kc:kc + 1], vrow[:, kc * P:(kc + 1) * P],
                                 ones_k1[:, 0:1], start=True, stop=True)
            rank4 = row_pool.tile([P, NKC], F32, tag="rank")
            for kc in range(NKC):
                scr = scr_pool.tile([P, L], U8, tag="scr")
                nc.vector.tensor_scalar(
                    scr[:], vbc[:], vt_ps[:, kc:kc + 1], None,
                    op0=OP.is_lt, op1=OP.add, accum_out=rank4[:, kc:kc + 1])
            out2 = outp.rearrange("b (h k2 p) e -> (b h) p k2 e", h=2, p=P)
            for h in range(2):
                maskh = row_pool.tile([P, 2], U8, tag=f"maskh{h}")
                nc.gpsimd.tensor_scalar(maskh[:], rank4[:, 2 * h:2 * h + 2],
                                        km1_bc[:, si, b:b + 1], None, op0=OP.is_le)
                nc.vector.copy_predicated(
                    et[:, 2 * h:2 * h + 2],
                    maskh[:].unsqueeze(2).to_broadcast([P, 2, E]),
                    memb_bc[:].unsqueeze(1).to_broadcast([P, 2, E]))
                nc.sync.dma_start(out=out2[b * 2 + h], in_=et[:, 2 * h:2 * h + 2])

        # 3-stage software pipeline: sums(r) | rank(r-1) | blend(r-2).
        # blend(r-2) is emitted before rank(r-1) so the DVE always has
        # ready work while PE finishes sums(r)/vt(r-1).
        state = {}
        nrows = len(rows)
        for r in range(nrows):
            state[r] = {}
            state[r]["vrow"], state[r]["vbc"], state[r]["et"] = emit_sums(r)
            if r == 0:
                # head: start the rank pipeline immediately (one-time ~0.7us
                # PE wait on the vrow copy, but DVE fills ~10us sooner)
                state[0]["mask"] = emit_rank(0, state[0]["vrow"], state[0]["vbc"])
            if r >= 2:
                emit_blend(r - 2, state[r - 2]["et"], state[r - 2]["mask"])
            if r >= 2:
                state[r - 1]["mask"] = emit_rank(
                    r - 1, state[r - 1]["vrow"], state[r - 1]["vbc"])
        # blend(14) is emitted before rank(15): its inputs are ready while
        # rank(15) still waits on the last (DMA-gated) attn row, so the DVE
        # keeps working through the drain
        emit_blend(nrows - 2, state[nrows - 2]["et"], state[nrows - 2]["mask"])
        emit_tail(nrows - 1, state[nrows - 1]["vrow"], state[nrows - 1]["vbc"],
                  state[nrows - 1]["et"])

    nc.compile()
    return nc


_NC_CACHE = None


def _get_nc() -> bass.Bass:
    global _NC_CACHE
    if _NC_CACHE is None:
        _NC_CACHE = _build()
    return _NC_CACHE


def _run(inputs, trace=False):
    nc = _get_nc()
    in_maps = []
    for c in range(N_CORES):
        sl = slice(c * B_LOC, (c + 1) * B_LOC)
        in_maps.append({
            "attn_a": np.ascontiguousarray(np.asarray(inputs["attn_a"])[sl]),
            "attn_b": np.ascontiguousarray(np.asarray(inputs["attn_b"])[sl]),
            "embed_a": np.ascontiguousarray(np.asarray(inputs["embed_a"])[sl]),
            "embed_b": np.ascontiguousarray(np.asarray(inputs["embed_b"])[sl]),
            "mask_embedding": np.asarray(inputs["mask_embedding"]),
            "a_padding_mask": np.ascontiguousarray(
                np.asarray(inputs["a_padding_mask"])[sl]).view(np.uint8),
            "b_padding_mask": np.ascontiguousarray(
                np.asarray(inputs["b_padding_mask"])[sl]).view(np.uint8),
        })
    res = run_bass_kernel_spmd(nc, in_maps, core_ids=list(range(N_CORES)), trace=trace)
    out_b = np.concatenate([res.results[c]["out_b"] for c in range(N_CORES)], axis=0)
    out_a = np.concatenate([res.results[c]["out_a"] for c in range(N_CORES)], axis=0)
    return (out_b, out_a), res


def kernel(**inputs):
    outs, _ = _run(inputs, trace=False)
    return outs
